# revision 12
# baseline (speedup 1.0000x reference)
"""Trainium2 Bass kernel for a dense transformer block (B=2, T=2048, C=1024, H=16).

Sharding across 8 NeuronCores (collective-minimal fp8 design):
  - NO AllGather: every core receives the FULL x pre-quantized to fp8
    (x8 = 8*x, 4MB) and replicates LayerNorm1 for all 4096 tokens,
    pipelined chunk-by-chunk with QKV+attention.  LN1 stats run as fp8
    DoubleRow ones-matmuls (sum and sum-of-squares); the x->fp8
    quantization error is subsumed by the h->fp8 cast the attention path
    already performs (measured end-to-end 2.9e-3 vs 2.8e-3 before).
  - Attention tensor-parallel over heads (2 heads/core).  Scores now run
    in fp8 DoubleRow: q/k are stored in a (pair, parity) layout
    [32 partitions x 2 x tokens] per head so the 64-deep head contraction
    becomes 32 partitions x 2 elements, halving PE columns per score
    tile.  QKV projection emits q/k in that layout directly via
    even/odd-column weight permutation (host side).
  - Softmax denominators: reciprocal moved off the DVE (3.3us each) to
    the ACT engine as exp(-ln(d)) -- the natural_log_exp table set is
    already resident for the attention exps.
  - AllToAll converts head-sharded y (fp8) to token-sharded y; Wp, LN2
    and the MLP are token-sharded (512 tokens/core).  W1 runs in fp8
    DoubleRow with h2 quantized to fp8 (the Gelu activation descales);
    W2 stays bf16: a numpy error model puts the total at 1.73e-2 vs the
    2e-2 gate, and adding W2/m fp8 would push past it.

Numerics: attention weights prescaled x32 into fp8e4 (TRN E4M3, max
240); the 1/32 folds into bias-add/activation ops.  The V path needs no
descale: the softmax-denominator ones-columns are 32 so the scale
cancels in the ratio.  rsqrt is exp(-0.5*ln(var+eps)) to stay in the
natural_log_exp ACT table set.  Causal masking: memset the fully-masked
prefix, triangle multiply on the 128-wide diagonal block.
"""

import sys

sys.path.insert(0, "/opt/trn_rl_repo")

import numpy as np
import ml_dtypes

import concourse.bass as bass
import concourse.bacc as bacc
import concourse.tile as tile
import concourse.mybir as mybir
from concourse import bass_utils

B, T, C, H = 2, 2048, 1024, 16
HD = C // H          # 64
FF = 4 * C           # 4096
EPS = 1e-5
NC = 8               # cores
P = 128
SH = (B * T) // NC   # 512 tokens per shard
KT = C // P          # 8 k-subtiles over C
FT = FF // P         # 32 ff tiles
TTILES = (B * T) // P  # 32 global 128-token tiles
CPB = T // SH        # 4 chunks per batch
WS = 32.0            # weight prescale into fp8
XS = 8.0             # x prescale into fp8
W1_FP8 = True
f32 = mybir.dt.float32
f32r = mybir.dt.float32r
bf16 = mybir.dt.bfloat16
fp8 = mybir.dt.float8e4
BF = ml_dtypes.bfloat16
F8 = ml_dtypes.float8_e4m3

_CACHE = {}


def _build(stub_collectives=False, loops=1):
    nc = bacc.Bacc("TRN2", target_bir_lowering=False, debug=False,
                   num_devices=1 if stub_collectives else NC)
    A = mybir.ActivationFunctionType
    OP = mybir.AluOpType
    DR = mybir.MatmulPerfMode.DoubleRow

    def dram_in(name, shape, dt):
        return nc.dram_tensor(name, shape, dt, kind="ExternalInput").ap()

    xF8 = dram_in("xF8", [P, NC, KT, SH], fp8)  # full x * XS, chunk-major
    xT = dram_in("xT", [P, KT, SH], f32)        # own token shard (residual)
    wqT = dram_in("wqT", [P, KT, P], fp8)       # [ki, ko, M], x32, M=64j+32h+dd
    wkT = dram_in("wkT", [P, KT, P], fp8)
    wvT = dram_in("wvT", [P, KT, P], fp8)       # M unpermuted
    wpT = dram_in("wpT", [P, KT, C], fp8)
    w1T = dram_in("w1T", [FT, P, KT, P], fp8 if W1_FP8 else bf16)
    w2T = dram_in("w2T", [KT, P, FT, P], bf16)
    bqk = dram_in("bqk", [P, 4], f32)   # [0:64, j]=bq pair j; [0:64, 2+j]=bk pair j
    bv = dram_in("bv", [1, P], f32)     # 32*bv slice (free-axis add)
    bp = dram_in("bp", [P, KT], f32)
    b1 = dram_in("b1", [P, FT], f32)
    b2 = dram_in("b2", [P, KT], f32)
    ln1w = dram_in("ln1w", [P, KT], f32)
    ln1b = dram_in("ln1b", [P, KT], f32)
    ln2w = dram_in("ln2w", [P, KT], f32)
    ln2b = dram_in("ln2b", [P, KT], f32)
    masks = dram_in("masks", [P, P], bf16)      # triangular: [i, j] = i <= j

    outT = nc.dram_tensor("outT", [C, SH], f32, kind="ExternalOutput").ap()

    rg = [list(range(NC))]

    with tile.TileContext(nc) as tc:
        with (
            tc.tile_pool(name="dram", bufs=1, space="DRAM") as dram,
            tc.tile_pool(name="const", bufs=1) as const,
            tc.tile_pool(name="persist", bufs=1) as persist,
            tc.tile_pool(name="temps", bufs=3) as temps,
        ):
          for _it in range(loops):
            a2a_in = dram.tile([NC * P, SH], fp8)
            a2a_out = dram.tile([NC * P, SH], fp8)

            ones_f = const.tile([P, P], f32)
            nc.vector.memset(ones_f[:], 1.0)
            ones_r = const.tile([P, P], f32r)
            nc.vector.tensor_copy(ones_r[:], ones_f[:])
            ones8 = const.tile([P, 2, P], fp8)
            nc.vector.memset(ones8[:], 1.0)
            eps_t = const.tile([P, 1], f32)
            nc.vector.memset(eps_t[:], EPS)
            ln8_t = const.tile([P, 1], f32)
            nc.vector.memset(ln8_t[:], float(np.log(1.0 / XS)))
            bqk_t = const.tile([P, 4], f32)
            nc.sync.dma_start(bqk_t[:], bqk[:])
            bv_rep = const.tile([P, P], f32)
            nc.gpsimd.dma_start(
                bv_rep[:],
                bass.AP(tensor=bv.tensor, offset=bv.offset, ap=[[0, P], [1, P]]),
            )
            bp_t = const.tile([P, KT], f32)
            nc.gpsimd.dma_start(bp_t[:], bp[:])
            b1_t = const.tile([P, FT], f32)
            nc.gpsimd.dma_start(b1_t[:], b1[:])
            b2_t = const.tile([P, KT], f32)
            nc.gpsimd.dma_start(b2_t[:], b2[:])
            lnp = {}
            for nm, ap in (("ln1w", ln1w), ("ln1b", ln1b), ("ln2w", ln2w), ("ln2b", ln2b)):
                t = const.tile([P, KT], f32, tag=nm)
                nc.gpsimd.dma_start(t[:], ap[:])
                lnp[nm] = t
            mask_t = const.tile([P, P], bf16)
            nc.gpsimd.dma_start(mask_t[:], masks[:])

            xT_sb = persist.tile([P, KT, SH], f32)
            for k in range(KT):
                nc.gpsimd.dma_start(xT_sb[:, k, :], xT[:, k, :])

            wp_sb = persist.tile([P, KT, C], fp8)

            # ------------ Phase 1: replicated LN1 + QKV + attention ---------
            with (
                tc.tile_pool(name="ph1", bufs=1) as ph1,
                tc.tile_pool(name="xstream", bufs=3) as xstream,
                tc.tile_pool(name="sqpool", bufs=2) as sqpool,
                tc.tile_pool(name="hstream", bufs=3) as hstream,
                tc.tile_pool(name="lnfin", bufs=2) as lnfin,
                tc.tile_pool(name="ppool", bufs=8) as ppool,
                tc.tile_pool(name="psum_s", bufs=2, space="PSUM") as psum_s,
                tc.tile_pool(name="psum_x", bufs=2, space="PSUM") as psum_x,
            ):
                wq_sb = ph1.tile([P, KT, P], fp8)
                nc.sync.dma_start(wq_sb[:], wqT[:])
                wk_sb = ph1.tile([P, KT, P], fp8)
                nc.sync.dma_start(wk_sb[:], wkT[:])
                wv_sb = ph1.tile([P, KT, P], fp8)
                nc.sync.dma_start(wv_sb[:], wvT[:])

                # q/k in DoubleRow pair layout: partition 32h+dd, slot j,
                # channel d = 2*dd + j of head h
                qT_dr = ph1.tile([P, NC, 2, SH], fp8)
                kT_dr = ph1.tile([P, NC, 2, SH], fp8)
                # v token-major, augmented with 64 WS-columns per head (the
                # x32 weight scale cancels in the softmax num/den ratio)
                v_aug = ph1.tile([P, TTILES, 4, HD], fp8)
                nc.vector.memset(v_aug[:, :, 1, :], WS)
                nc.vector.memset(v_aug[:, :, 3, :], WS)
                yT_sb = ph1.tile([P, NC, SH], fp8)

                def ln1(g):
                    """LayerNorm chunk g (512 tokens, all channels) from fp8
                    full-x; emits h_g fp8 in SBUF.  Stats via fp8-DR
                    ones-matmuls; x carries scale XS=8, squares 4x^2."""
                    xg = xstream.tile([P, KT, SH], fp8, tag="xg")
                    xeng = nc.sync if g % 2 == 0 else nc.gpsimd
                    xeng.dma_start(xg[:], xF8[:, g])
                    sq = sqpool.tile([P, KT, SH], fp8, tag="sq")
                    # sq = Square(xg/4) = 4*x^2  (xg = 8x; max ~130, in range)
                    nc.scalar.activation(sq[:, 0:4, :], xg[:, 0:4, :], A.Square, scale=2.0 / XS)
                    nc.scalar.activation(sq[:, 4:8, :], xg[:, 4:8, :], A.Square, scale=2.0 / XS)
                    s12 = psum_x.tile([P, 2, SH], f32, tag="px")
                    for kp in range(KT // 2):
                        ks = slice(2 * kp, 2 * kp + 2)
                        st, sp = kp == 0, kp == KT // 2 - 1
                        nc.tensor.matmul(s12[:, 0, :], ones8[:], xg[:, ks, :], start=st, stop=sp, perf_mode=DR)
                        nc.tensor.matmul(s12[:, 1, :], ones8[:], sq[:, ks, :], start=st, stop=sp, perf_mode=DR)
                    # fin[:,0] = XS*mean, fin[:,1] = rsqrt(var+eps)/XS
                    fin = lnfin.tile([P, 2, SH], f32, tag="fin")
                    nc.vector.tensor_scalar_mul(fin[:, 0, :], s12[:, 0, :], 1.0 / C)
                    # s2 = sum 4x^2; XS^2/4 / C turns it into XS^2*E[x^2]
                    v64 = temps.tile([P, SH], f32, tag="v64")
                    nc.vector.tensor_scalar_mul(v64[:], s12[:, 1, :], (XS * XS / 4.0) / C)
                    msq = temps.tile([P, SH], f32, tag="msq")
                    nc.gpsimd.tensor_tensor(msq[:], fin[:, 0, :], fin[:, 0, :], OP.mult)
                    nc.vector.tensor_tensor(v64[:], v64[:], msq[:], OP.subtract)
                    # v64 = XS^2*var; rsqrt via ln+exp in the exp table set
                    nc.scalar.activation(v64[:], v64[:], A.Ln, bias=eps_t[:], scale=1.0 / (XS * XS))
                    nc.scalar.activation(fin[:, 1, :], v64[:], A.Exp, scale=-0.5, bias=ln8_t[:])
                    h_g = hstream.tile([P, KT, SH], fp8, tag="hg")
                    for k in range(KT):
                        eng = nc.vector if k % 2 == 0 else nc.gpsimd
                        t = temps.tile([P, SH], f32, tag="lnt")
                        eng.tensor_tensor(t[:], xg[:, k, :], fin[:, 0, :], OP.subtract)
                        eng.tensor_tensor(t[:], t[:], fin[:, 1, :], OP.mult)
                        eng.tensor_scalar(h_g[:, k, :], t[:], lnp["ln1w"][:, k : k + 1], lnp["ln1b"][:, k : k + 1], OP.mult, OP.add)
                    return h_g

                def qkv(g, h_g):
                    # q/k for this chunk in pair layout (M=64 even/odd
                    # column split), v in token-major fp8
                    pq = psum_s.tile([P, 2, SH], f32, tag="spair")
                    pk = psum_s.tile([P, 2, SH], f32, tag="spair")
                    for kp in range(KT // 2):
                        ks = slice(2 * kp, 2 * kp + 2)
                        st, sp = kp == 0, kp == KT // 2 - 1
                        for j in (0, 1):
                            nc.tensor.matmul(pq[0:64, j, :], wq_sb[:, ks, 64 * j : 64 * j + 64], h_g[:, ks, :], start=st, stop=sp, perf_mode=DR)
                            nc.tensor.matmul(pk[0:64, j, :], wk_sb[:, ks, 64 * j : 64 * j + 64], h_g[:, ks, :], start=st, stop=sp, perf_mode=DR)
                    for j in (0, 1):
                        nc.vector.tensor_scalar(qT_dr[0:64, g, j, :], pq[0:64, j, :], 1.0 / WS, bqk_t[0:64, j : j + 1], OP.mult, OP.add)
                        nc.vector.tensor_scalar(kT_dr[0:64, g, j, :], pk[0:64, j, :], 1.0 / WS, bqk_t[0:64, 2 + j : 3 + j], OP.mult, OP.add)
                    pv2 = psum_s.tile([P, 2, SH], f32, tag="spair")
                    for jj in (0, 2, 1, 3):   # alternate PSUM banks
                        j = 4 * g + jj
                        psv = pv2[:, jj // 2, (jj % 2) * P : (jj % 2) * P + P]
                        for kp in range(KT // 2):
                            ks = slice(2 * kp, 2 * kp + 2)
                            nc.tensor.matmul(
                                psv,
                                h_g[:, ks, jj * P : (jj + 1) * P],
                                wv_sb[:, ks, :],
                                start=(kp == 0), stop=(kp == KT // 2 - 1),
                                perf_mode=DR,
                            )
                        # v_stored = 32*(h@WvT) + 32*bv; fp8 out
                        nc.vector.tensor_tensor(
                            v_aug[:, j, 0::2, :],
                            psv.rearrange("p (hh x) -> p hh x", x=HD),
                            bv_rep.rearrange("p (hh x) -> p hh x", x=HD),
                            OP.add,
                        )

                def attn(g):
                    # attention for one (batch, q-chunk); 2 heads per core
                    b, qc = g // CPB, g % CPB
                    n_kt = 4 * (qc + 1)
                    ya = psum_x.tile([P, 2, SH], f32, tag="px")
                    for kp in range(n_kt // 2):
                        kt0, kt1 = 2 * kp, 2 * kp + 1
                        s0 = psum_s.tile([P, 2, SH], f32, tag="spair")
                        s1 = psum_s.tile([P, 2, SH], f32, tag="spair")
                        for i, kt in enumerate((kt0, kt1)):
                            ch = b * CPB + kt // 4
                            ksl = slice((kt % 4) * P, (kt % 4 + 1) * P)
                            nc.tensor.matmul(s0[:, i, :], kT_dr[0:32, ch, :, ksl], qT_dr[0:32, g, :, :], start=True, stop=True, perf_mode=DR)
                            nc.tensor.matmul(s1[:, i, :], kT_dr[32:64, ch, :, ksl], qT_dr[32:64, g, :, :], start=True, stop=True, perf_mode=DR)
                        p0 = ppool.tile([P, 2, SH], fp8, tag="pt")
                        p1 = ppool.tile([P, 2, SH], fp8, tag="pt")
                        nc.scalar.activation(p0[:], s0[:], A.Exp, scale=1.0 / np.sqrt(HD))
                        nc.scalar.activation(p1[:], s1[:], A.Exp, scale=1.0 / np.sqrt(HD))
                        for i, kt in enumerate((kt0, kt1)):
                            d = kt - 4 * qc
                            if d >= 0:
                                # prefix q-cols fully masked; triangle only on
                                # the 128-wide diagonal block
                                for pt in (p0, p1):
                                    if d > 0:
                                        nc.gpsimd.memset(pt[:, i, 0 : P * d], 0.0)
                                    nc.vector.tensor_mul(
                                        pt[:, i, P * d : P * d + P],
                                        pt[:, i, P * d : P * d + P], mask_t[:])
                        j0 = 16 * b + kt0
                        nc.tensor.matmul(ya[:, 0, :], v_aug[:, j0 : j0 + 2, 0:2, :], p0[:], start=(kp == 0), stop=(kp == n_kt // 2 - 1), perf_mode=DR)
                        nc.tensor.matmul(ya[:, 1, :], v_aug[:, j0 : j0 + 2, 2:4, :], p1[:], start=(kp == 0), stop=(kp == n_kt // 2 - 1), perf_mode=DR)
                    # softmax denominators: reciprocal as exp(-ln d) on ACT
                    # (stays in the natural_log_exp table set)
                    for hh in (0, 1):
                        rec = temps.tile([P, SH], f32, tag="rec")
                        nc.scalar.activation(rec[HD:P, :], ya[HD:P, hh, :], A.Ln)
                        nc.scalar.activation(rec[HD:P, :], rec[HD:P, :], A.Exp, scale=-1.0)
                        nc.vector.tensor_tensor(yT_sb[HD * hh : HD * hh + HD, g, :], ya[0:HD, hh, :], rec[HD:P, :], OP.mult)
                    nc.sync.dma_start(a2a_in[g * P : (g + 1) * P, :], yT_sb[:, g, :])

                # pipeline: LN chunk ahead of QKV, attention follows; heavy
                # chunks early, lightest chunk last so the pre-A2A tail is
                # short
                h0 = ln1(0); h1 = ln1(1)
                qkv(0, h0); qkv(1, h1)
                attn(0)
                h2 = ln1(2); qkv(2, h2)
                attn(1)
                h3 = ln1(3); qkv(3, h3)
                attn(3)
                h4 = ln1(4); qkv(4, h4)
                h5 = ln1(5); qkv(5, h5)
                attn(2)
                h6 = ln1(6); qkv(6, h6)
                attn(5)
                h7 = ln1(7); qkv(7, h7)
                nc.gpsimd.dma_start(wp_sb[:], wpT[:])
                attn(7); attn(6); attn(4)

            # ---------------- Phase 2: A2A + Wp + LN2 + MLP -----------------
            with (
                tc.tile_pool(name="ph3", bufs=1) as ph3,
                tc.tile_pool(name="w1p", bufs=3) as w1p,
                tc.tile_pool(name="w2p", bufs=2) as w2p,
                tc.tile_pool(name="psum_t", bufs=8, space="PSUM") as psum_t,
            ):
                if stub_collectives:
                    nc.sync.dma_start(a2a_out[:], a2a_in[:])
                else:
                    nc.gpsimd.collective_compute(
                        "AllToAll", mybir.AluOpType.bypass, replica_groups=rg,
                        ins=[a2a_in.opt()], outs=[a2a_out.opt()],
                    )
                # y in k-tile pairs so Wp can start on the first pair
                y_pair = []
                for jp in range(KT // 2):
                    yp = ph3.tile([P, 2, SH], fp8, tag=f"yp{jp}")
                    for i in range(2):
                        k = 2 * jp + i
                        eng = nc.sync if k % 2 == 0 else nc.gpsimd
                        eng.dma_start(yp[:, i, :], a2a_out[k * P : (k + 1) * P, :])
                    y_pair.append(yp)

                def ln_stats_feed(s1, s2, x_ap, k):
                    """Feed one [P, SH] fp32 tile into the LN stat
                    accumulators via all-ones f32r matmuls (sum over the C
                    partition axis, result broadcast to every partition)."""
                    sq = temps.tile([P, SH], f32r, tag="ln_sq")
                    nc.vector.tensor_mul(sq[:], x_ap, x_ap)
                    nc.tensor.matmul(s1[:], ones_r[:], x_ap, start=(k == 0), stop=(k == KT - 1))
                    nc.tensor.matmul(s2[:], ones_r[:], sq[:], start=(k == 0), stop=(k == KT - 1))

                x2T = ph3.tile([P, KT, SH], f32r)
                ls1 = psum_t.tile([P, SH], f32, tag="pst")
                ls2 = psum_t.tile([P, SH], f32, tag="pst")
                for mp in range(KT // 2):
                    # two interleaved accumulation chains (alternate banks)
                    pa = psum_t.tile([P, SH], f32, tag="pst")
                    pb = psum_t.tile([P, SH], f32, tag="pst")
                    m0, m1 = 2 * mp, 2 * mp + 1
                    for kp in range(KT // 2):
                        nc.tensor.matmul(pa[:], wp_sb[:, 2 * kp : 2 * kp + 2, m0 * P : (m0 + 1) * P], y_pair[kp][:], start=(kp == 0), stop=(kp == KT // 2 - 1), perf_mode=DR)
                        nc.tensor.matmul(pb[:], wp_sb[:, 2 * kp : 2 * kp + 2, m1 * P : (m1 + 1) * P], y_pair[kp][:], start=(kp == 0), stop=(kp == KT // 2 - 1), perf_mode=DR)
                    for m, ps in ((m0, pa), (m1, pb)):
                        t = temps.tile([P, SH], f32, tag="ev")
                        nc.vector.tensor_scalar(t[:], ps[:], 1.0 / WS, bp_t[:, m : m + 1], OP.mult, OP.add)
                        nc.gpsimd.tensor_tensor(x2T[:, m, :], t[:], xT_sb[:, m, :], OP.add)
                        ln_stats_feed(ls1, ls2, x2T[:, m, :], m)

                h2T = ph3.tile([P, KT, SH], fp8 if W1_FP8 else bf16)

                # LN2 finalize (f32 path)
                mean = temps.tile([P, SH], f32, tag="ln_mean")
                nc.vector.tensor_scalar_mul(mean[:], ls1[:], 1.0 / C)
                var = temps.tile([P, SH], f32, tag="ln_var")
                nc.vector.tensor_scalar_mul(var[:], ls2[:], 1.0 / C)
                msq = temps.tile([P, SH], f32, tag="ln_t")
                nc.vector.tensor_mul(msq[:], mean[:], mean[:])
                nc.vector.tensor_sub(var[:], var[:], msq[:])
                nc.scalar.activation(var[:], var[:], A.Ln, bias=eps_t[:])
                rs = temps.tile([P, SH], f32, tag="ln_rs")
                nc.scalar.activation(rs[:], var[:], A.Exp, scale=-0.5)
                for k in range(KT):
                    t = temps.tile([P, SH], f32, tag="ln_t")
                    nc.vector.tensor_sub(t[:], x2T[:, k, :], mean[:])
                    nc.vector.tensor_mul(t[:], t[:], rs[:])
                    nc.vector.tensor_scalar(h2T[:, k, :], t[:], lnp["ln2w"][:, k : k + 1], lnp["ln2b"][:, k : k + 1], OP.mult, OP.add)

                mT = ph3.tile([P, FT, SH], bf16)
                for fp_ in range(FT // 2):
                    f0, f1 = 2 * fp_, 2 * fp_ + 1
                    w1a = w1p.tile([P, KT, P], fp8 if W1_FP8 else bf16, tag="w1a")
                    nc.sync.dma_start(w1a[:], w1T[f0])
                    w1b = w1p.tile([P, KT, P], fp8 if W1_FP8 else bf16, tag="w1b")
                    nc.sync.dma_start(w1b[:], w1T[f1])
                    pa = psum_t.tile([P, SH], f32, tag="pst")
                    pb = psum_t.tile([P, SH], f32, tag="pst")
                    if W1_FP8:
                        for kp in range(KT // 2):
                            ks = slice(2 * kp, 2 * kp + 2)
                            st, sp = kp == 0, kp == KT // 2 - 1
                            nc.tensor.matmul(pa[:], w1a[:, ks, :], h2T[:, ks, :], start=st, stop=sp, perf_mode=DR)
                            nc.tensor.matmul(pb[:], w1b[:, ks, :], h2T[:, ks, :], start=st, stop=sp, perf_mode=DR)
                        gsc = 1.0 / WS
                    else:
                        for k in range(KT):
                            nc.tensor.matmul(pa[:], w1a[:, k, :], h2T[:, k, :], start=(k == 0), stop=(k == KT - 1))
                            nc.tensor.matmul(pb[:], w1b[:, k, :], h2T[:, k, :], start=(k == 0), stop=(k == KT - 1))
                        gsc = 1.0
                    nc.scalar.activation(mT[:, f0, :], pa[:], A.Gelu, bias=b1_t[:, f0 : f0 + 1], scale=gsc)
                    nc.scalar.activation(mT[:, f1, :], pb[:], A.Gelu, bias=b1_t[:, f1 : f1 + 1], scale=gsc)

                for mp in range(KT // 2):
                    m0, m1 = 2 * mp, 2 * mp + 1
                    w2a = w2p.tile([P, FT, P], bf16, tag="w2a")
                    nc.sync.dma_start(w2a[:], w2T[m0])
                    w2b = w2p.tile([P, FT, P], bf16, tag="w2b")
                    nc.sync.dma_start(w2b[:], w2T[m1])
                    pa = psum_t.tile([P, SH], f32, tag="pst")
                    pb = psum_t.tile([P, SH], f32, tag="pst")
                    for k in range(FT):
                        nc.tensor.matmul(pa[:], w2a[:, k, :], mT[:, k, :], start=(k == 0), stop=(k == FT - 1))
                        nc.tensor.matmul(pb[:], w2b[:, k, :], mT[:, k, :], start=(k == 0), stop=(k == FT - 1))
                    for m, ps in ((m0, pa), (m1, pb)):
                        of = temps.tile([P, SH], f32, tag="ev")
                        nc.vector.tensor_scalar(of[:], ps[:], b2_t[:, m : m + 1], None, OP.add)
                        of2 = temps.tile([P, SH], f32, tag="ev2")
                        nc.gpsimd.tensor_tensor(of2[:], of[:], x2T[:, m, :], OP.add)
                        nc.sync.dma_start(
                            outT.rearrange("(ko ki) s -> ki ko s", ki=P)[:, m, :],
                            of2[:])

    nc.compile()
    return nc


def _prep_inputs(inputs):
    x = np.asarray(inputs["x"], np.float32)
    x2d = np.ascontiguousarray(x.reshape(B * T, C))
    xT_full = np.ascontiguousarray(x2d.T)  # [C, B*T]

    Wq = np.asarray(inputs["Wq"], np.float32)
    Wk = np.asarray(inputs["Wk"], np.float32)
    Wv = np.asarray(inputs["Wv"], np.float32)
    Wp = np.asarray(inputs["Wp"], np.float32)
    W1 = np.asarray(inputs["W1"], np.float32)
    W2 = np.asarray(inputs["W2"], np.float32)

    def to8(a):
        return np.clip(a * WS, -240.0, 240.0).astype(F8)

    def block_k(a, conv=None):
        # [KO*P, M] -> [P, KO, M]   (row r = 128*ko + ki)
        ko = a.shape[0] // P
        out = np.ascontiguousarray(a.reshape(ko, P, a.shape[1]).transpose(1, 0, 2))
        return conv(out) if conv else out

    def tobf(a):
        return a.astype(BF)

    # full x, fp8 x8, chunk-major: [P, NC, KT, SH]
    xF8 = np.ascontiguousarray(
        np.clip(xT_full * XS, -240.0, 240.0)
        .astype(F8)
        .reshape(KT, P, NC, SH)
        .transpose(1, 2, 0, 3)
    )

    wpT = block_k(Wp.T, to8)                              # [P, KT, C]
    w1conv = to8 if W1_FP8 else tobf
    w1T_f = W1.T                                          # [C, FF]
    w1T = np.ascontiguousarray(
        np.stack([block_k(w1T_f[:, f * P : (f + 1) * P], w1conv) for f in range(FT)])
    )                                                     # [FT, P, KT, P]
    w2T_f = W2.T                                          # [FF, C]
    w2T = np.ascontiguousarray(
        np.stack([block_k(w2T_f[:, m * P : (m + 1) * P], tobf) for m in range(KT)])
    )                                                     # [KT, P, FT, P]

    def pack_pcol(v, nt):  # [nt*P] -> [P, nt]
        return np.ascontiguousarray(np.asarray(v, np.float32).reshape(nt, P).T)

    bp = pack_pcol(inputs["bp"], KT)
    b1 = pack_pcol(inputs["b1"], FT)
    b2 = pack_pcol(inputs["b2"], KT)
    ln1w = pack_pcol(inputs["ln1_w"], KT)
    ln1b = pack_pcol(inputs["ln1_b"], KT)
    ln2w = pack_pcol(inputs["ln2_w"], KT)
    ln2b = pack_pcol(inputs["ln2_b"], KT)

    i_idx = np.arange(P)[:, None]
    j_idx = np.arange(P)[None, :]
    masks = (i_idx <= j_idx).astype(BF)   # [P, P] triangular

    bq = np.asarray(inputs["bq"], np.float32)
    bk = np.asarray(inputs["bk"], np.float32)
    bvv = np.asarray(inputs["bv"], np.float32)

    # even/odd column permutation for the q/k pair layout:
    # m = 64*j + 32*h + dd  <->  channel 64*h + 2*dd + j
    perm = np.array(
        [64 * h + 2 * dd + j for j in (0, 1) for h in (0, 1) for dd in range(32)]
    )

    in_maps = []
    for c in range(NC):
        rs = slice(P * c, P * (c + 1))
        bq_c, bk_c = bq[rs][perm], bk[rs][perm]
        bqk_c = np.zeros((P, 4), np.float32)
        bqk_c[0:64, 0] = bq_c[0:64]
        bqk_c[0:64, 1] = bq_c[64:128]
        bqk_c[0:64, 2] = bk_c[0:64]
        bqk_c[0:64, 3] = bk_c[64:128]
        m = {
            "xF8": xF8,
            "xT": block_k(xT_full[:, SH * c : SH * (c + 1)]).astype(np.float32),
            "wqT": block_k(Wq[rs, :][perm, :].T, to8),
            "wkT": block_k(Wk[rs, :][perm, :].T, to8),
            "wvT": block_k(Wv[rs, :].T, to8),
            "wpT": wpT,
            "w1T": w1T,
            "w2T": w2T,
            "bqk": bqk_c,
            "bv": np.ascontiguousarray((WS * bvv[rs])[None, :]),
            "bp": bp, "b1": b1, "b2": b2,
            "ln1w": ln1w, "ln1b": ln1b, "ln2w": ln2w, "ln2b": ln2b,
            "masks": masks,
        }
        in_maps.append(m)
    return in_maps


def kernel(**inputs):
    if "nc" not in _CACHE:
        _CACHE["nc"] = _build()
    nc = _CACHE["nc"]
    in_maps = _prep_inputs(inputs)
    res = bass_utils.run_bass_kernel_spmd(nc, in_maps, core_ids=list(range(NC)))
    out2d = np.empty((B * T, C), np.float32)
    for c in range(NC):
        out2d[SH * c : SH * (c + 1), :] = res.results[c]["outT"].T
    return out2d.reshape(B, T, C)
